# revision 21
# baseline (speedup 1.0000x reference)
"""Mixtral-style GQA attention block on 8 Trainium2 NeuronCores.

Tensor-parallel over heads: core c owns q-heads [4c..4c+4) and kv-head c.
All matmuls bf16 (1 cycle/row), fp32 PSUM accumulation.

v2: chunk-pipelined single phase. For each 512-token chunk c:
  qkv(c) (6 slots x 32 kc, PE-dense) -> rope (DVE: pair-interleaved head dim
  so rotate-half is a stream_shuffle, no DMA) -> attention(c) in head-pairs
  (scores 2-at-a-time into a 2-bank PSUM group, one batched exp per j,
  softmax denominator accumulated on DVE) -> per-pair normalization ->
  AllGather(c) -> o_proj(c) slotted into the NEXT chunk's attention bubbles.
The Tile list scheduler interleaves qkv(c+1)/o_proj(c-1) matmuls into
attention's scalar-exp bubbles, keeping the PE streaming end-to-end.

DMA queues: bulk loads (weights, hid) on sync/HWDGE; latency-critical small
DMAs (v transposes, dnr rows, avn stores, out stores) on scalar/HWDGE;
AG-gated o_proj operand loads on gpsimd/SWDGE so they can't head-of-line
block either HWDGE ring. hid is host-packed partition-major per chunk so
each load is 4 big-line (8KB/partition) DMAs.

Model dims (hardcoded): T=2048, HIDDEN=4096, H=32, KV=8, D=128.
"""

import ml_dtypes
import numpy as np

import concourse.mybir as mybir
import concourse.tile as tile
from concourse import bacc
from concourse.bass_utils import run_bass_kernel_spmd

# ---- problem dims ----
T = 2048
HIDDEN = 4096
H = 32
KV = 8
D = 128
THETA = 10000.0
SCALE = D ** -0.5

CORES = 8
QH = H // CORES            # 4 q heads per core
SLOTS = QH + 2             # k, v, q0..q3 head-major slots
LOCAL = QH * D             # 512: per-core attention output dims
P = 128
NCH = T // 512             # 4 token chunks of 512
KCH = HIDDEN // P          # 32 contraction chunks
TT = T // P                # 16 token tiles of 128
GK = 8                     # kc per hid DMA group
NG = KCH // GK             # 4 hid groups per chunk

F32 = mybir.dt.float32
BF = mybir.dt.bfloat16
EXP = mybir.ActivationFunctionType.Exp
BF_NP = ml_dtypes.bfloat16

# swap adjacent partitions within each 32-lane quadrant
SWAP_MASK = [i ^ 1 for i in range(32)]


def build_nc():
    nc = bacc.Bacc(num_devices=CORES)

    # ---- per-core I/O (host pre-packs bf16 + stationary-major weights) ----
    # hidP[c, p, kc*512+t] = hidden_states[c*512+t, kc*128+p]
    hidP = nc.declare_dram_parameter("hidP", [NCH, P, KCH * 512], BF,
                                     isOutput=False)
    w_qkvT = nc.declare_dram_parameter("w_qkvT", [SLOTS * P, KCH * P], BF,
                                       isOutput=False)
    w_oT = nc.declare_dram_parameter("w_oT", [QH * P, KCH * P], BF,
                                     isOutput=False)
    cosT = nc.declare_dram_parameter("cosT", [P, T], BF, isOutput=False)
    sinT = nc.declare_dram_parameter("sinT", [P, T], BF, isOutput=False)
    outT = nc.declare_dram_parameter("outT", [LOCAL, T], F32, isOutput=True)

    # ---- consts ----
    tri_c = nc.inline_tensor(
        np.triu(np.ones((P, P), dtype=np.float32)).astype(BF_NP), name="tri_c")
    eye_c = nc.inline_tensor(
        np.eye(P, dtype=np.float32).astype(BF_NP), name="eye_c")
    ones_col_c = nc.inline_tensor(
        np.ones((P, 1), dtype=np.float32).astype(BF_NP), name="ones_col_c")
    ones_row_c = nc.inline_tensor(
        np.ones((1, P), dtype=np.float32).astype(BF_NP), name="ones_row_c")

    # ---- collective bounce buffers (chunk-major, bf16) ----
    ag_in = nc.dram_tensor("ag_in", [NCH, 2, 2 * P, 512], BF)
    ag_outF = nc.dram_tensor("ag_outF", [NCH - 1, CORES * 4 * P, 512], BF,
                             addr_space="Shared")
    ag_out = nc.dram_tensor("ag_out", [2, CORES * 2 * P, 512], BF,
                            addr_space="Shared")
    agw_in = nc.dram_tensor("agw_in", [1, 64], BF)
    agw_out = nc.dram_tensor("agw_out", [CORES, 64], BF, addr_space="Shared")

    with tile.TileContext(nc) as tc:
        with tc.tile_pool(name="const", bufs=1) as cpool, \
             tc.tile_pool(name="wq", bufs=1) as wq_pool, \
             tc.tile_pool(name="wo", bufs=1) as wo_pool, \
             tc.tile_pool(name="cs", bufs=1) as cs_pool, \
             tc.tile_pool(name="hid", bufs=1) as hid_pool, \
             tc.tile_pool(name="qk", bufs=1) as qk_pool, \
             tc.tile_pool(name="att", bufs=1) as att_pool, \
             tc.tile_pool(name="ssum", bufs=1) as s_pool, \
             tc.tile_pool(name="sm", bufs=1) as sm_pool, \
             tc.tile_pool(name="rp", bufs=1) as rp_pool, \
             tc.tile_pool(name="agp", bufs=1) as ag_pool, \
             tc.tile_pool(name="oo", bufs=1) as oo_pool, \
             tc.tile_pool(name="scp", bufs=1, space="PSUM") as scp_ps, \
             tc.tile_pool(name="avp", bufs=1, space="PSUM") as av_ps, \
             tc.tile_pool(name="gp", bufs=1, space="PSUM") as gp_ps:

            # ---- initial bulk DMAs (sync/HWDGE queue, consumption order) --
            # warm up the collective path before anything else on PL
            nc.gpsimd.collective_compute(
                "AllGather", mybir.AluOpType.bypass,
                replica_groups=[list(range(CORES))],
                ins=[agw_in[:]], outs=[agw_out[:]])

            wms = [wq_pool.tile([P, KCH * P], BF, tag=f"wm{m}",
                                name=f"wm{m}") for m in range(SLOTS)]

            def issue_wm(m, parts=2, eng=None):
                eng = eng or nc.sync
                step = KCH * P // parts
                for s in range(parts):
                    eng.dma_start(wms[m][:, s * step:(s + 1) * step],
                                  w_qkvT[m * P:(m + 1) * P,
                                         s * step:(s + 1) * step])

            hts = {}     # (chunk, group) -> tile

            def issue_hid(c, g0=0, halves=False):
                for g in range(g0, NG):
                    ht = hid_pool.tile([P, GK * 512], BF, tag=f"ht{g}",
                                       name=f"ht{c}_{g}", bufs=1)
                    if halves:
                        hw = GK * 512 // 2
                        nc.sync.dma_start(
                            ht[:, :hw], hidP[c, :, g * GK * 512:
                                             g * GK * 512 + hw])
                        nc.sync.dma_start(
                            ht[:, hw:], hidP[c, :, g * GK * 512 + hw:
                                             (g + 1) * GK * 512])
                    else:
                        nc.sync.dma_start(
                            ht[:], hidP[c, :,
                                        g * GK * 512:(g + 1) * GK * 512])
                    hts[(c, g)] = ht

            issue_wm(0, parts=4)
            issue_hid(0, g0=0, halves=True)
            issue_wm(1)
            cos_sb = cs_pool.tile([P, T], BF, tag="cos")
            sin_sb = cs_pool.tile([P, T], BF, tag="sin")
            nc.sync.dma_start(cos_sb[:], cosT[:, :])
            nc.sync.dma_start(sin_sb[:], sinT[:, :])
            tri_sb = cpool.tile([P, P], BF, tag="tri")
            nc.sync.dma_start(tri_sb[:], tri_c[:, :])
            onc1_sb = cpool.tile([P, 1], BF, tag="onc1")
            nc.sync.dma_start(onc1_sb[:], ones_col_c[:, :])
            onr_sb = cpool.tile([1, P], BF, tag="onr")
            nc.sync.dma_start(onr_sb[:], ones_row_c[:, :])
            eye_sb = cpool.tile([P, P], BF, tag="eye")
            nc.sync.dma_start(eye_sb[:], eye_c[:, :])
            for m in range(2, SLOTS):
                issue_wm(m, eng=nc.gpsimd)
            wo_sb = wo_pool.tile([P, QH, KCH * P], BF, tag="wo")

            # ---- persistent attention operands ----
            k16 = qk_pool.tile([P, T], BF, tag="k", name="k16")
            vtok = {}

            def rope_to(dst, psq, c):
                """dst[:, :512] = rope(psq) in the pair-interleaved basis."""
                t0 = c * 512
                xs = rp_pool.tile([P, 512], BF, tag="xs", name="xs", bufs=2)
                nc.scalar.copy(xs[:], psq[:])
                xr = rp_pool.tile([P, 512], BF, tag="xr", name="xr", bufs=2)
                nc.vector.stream_shuffle(xr[:], xs[:], mask=SWAP_MASK)
                tcs = rp_pool.tile([P, 512], BF, tag="tc", name="tc", bufs=2)
                nc.vector.tensor_mul(tcs[:], xs[:], cos_sb[:, t0:t0 + 512])
                tsn = rp_pool.tile([P, 512], BF, tag="ts", name="ts", bufs=1)
                nc.vector.tensor_mul(tsn[:], xr[:], sin_sb[:, t0:t0 + 512])
                nc.vector.tensor_add(dst, tcs[:], tsn[:])

            def qkv_chunk(c):
                q16c = []
                for m in range(SLOTS):
                    psq = gp_ps.tile([P, 512], F32, tag="gp", name="psq",
                                     bufs=2)
                    for kc in range(KCH):
                        g, o = kc // GK, (kc % GK) * 512
                        nc.tensor.matmul(
                            psq[:], wms[m][:, kc * P:(kc + 1) * P],
                            hts[(c, g)][:, o:o + 512],
                            start=(kc == 0), stop=(kc == KCH - 1))
                    if m == 0:
                        rope_to(k16[:, c * 512:(c + 1) * 512], psq, c)
                    elif m == 1:
                        vtmp = rp_pool.tile([P, 512], BF, tag="vtmp",
                                            name="vtmp", bufs=1)
                        nc.scalar.copy(vtmp[:], psq[:])
                        vt_ps = gp_ps.tile([P, 4 * P], BF, tag="gp",
                                           name="vt_ps", bufs=2)
                        for jj in range(4):
                            nc.tensor.transpose(
                                vt_ps[:, jj * P:(jj + 1) * P],
                                vtmp[:, jj * P:(jj + 1) * P], eye_sb[:])
                        vt_c = qk_pool.tile([P, 4 * P], BF, tag=f"vtc{c}",
                                            name=f"vtc{c}")
                        nc.vector.tensor_copy(vt_c[:], vt_ps[:])
                        for jj in range(4):
                            vtok[c * 4 + jj] = vt_c[:, jj * P:(jj + 1) * P]
                        if c == 0:
                            issue_hid(1)
                    else:
                        qt = qk_pool.tile([P, 512], BF, tag=f"q{m - 2}",
                                          name=f"q{m - 2}_{c}", bufs=2)
                        rope_to(qt[:], psq, c)
                        q16c.append(qt)
                return q16c

            def norm_pair(c, pair, S2, av):
                """softmax-normalize the pair's AV psum, store to ag_in."""
                s16 = sm_pool.tile([P, 2, 512], BF, tag="s16", name="s16",
                                   bufs=1)
                nc.vector.tensor_copy(s16[:], S2[:])
                # both heads' denominators as one partition-0 row [1, 1024]
                dnp = gp_ps.tile([P, 512], F32, tag="gp", name="dnp", bufs=2)
                dnp2 = gp_ps.tile([P, 512], F32, tag="gp", name="dnp2",
                                  bufs=2)
                nc.tensor.matmul(dnp[0:1, :], onc1_sb[:], s16[:, 0, :],
                                 start=True, stop=True)
                nc.tensor.matmul(dnp2[0:1, :], onc1_sb[:], s16[:, 1, :],
                                 start=True, stop=True)
                rc = sm_pool.tile([1, 2, 512], F32, tag="rc", bufs=1)
                nc.vector.reciprocal_approx_fast(rc[:, 0, :], dnp[0:1, :])
                nc.vector.reciprocal_approx_fast(rc[:, 1, :], dnp2[0:1, :])
                rc16 = sm_pool.tile([1, 2, 512], BF, tag="rc16", bufs=1)
                nc.scalar.copy(rc16[:], rc[:])
                for i in range(2):
                    bcp = gp_ps.tile([P, 512], F32, tag="gp", name="bcp",
                                     bufs=2)
                    nc.tensor.matmul(bcp[:], onr_sb[:], rc16[:, i, :],
                                     start=True, stop=True)
                    bc_sb = sm_pool.tile([P, 512], F32, tag="bc_sb", bufs=1)
                    nc.scalar.copy(bc_sb[:], bcp[:])
                    avn = sm_pool.tile([P, 512], BF, tag="avn", bufs=1)
                    nc.vector.tensor_mul(avn[:], av[i][:], bc_sb[:])
                    nc.scalar.dma_start(ag_in[c, pair, i * P:(i + 1) * P, :],
                                        avn[:])
                if c == NCH - 1:
                    nc.gpsimd.collective_compute(
                        "AllGather", mybir.AluOpType.bypass,
                        replica_groups=[list(range(CORES))],
                        ins=[ag_in[c, pair]], outs=[ag_out[pair]])

            def att_pair(c, pair, q16c, interleave=None):
                t0 = c * 512
                jmax = 4 * c + 3
                S2 = [None]
                atts = {}

                def scores(j):
                    toff = max(t0, j * P)
                    w = t0 + 512 - toff
                    o = toff - t0
                    scp = scp_ps.tile([P, 2, 512], F32, tag="scp",
                                      name="scp", bufs=2)
                    for i in range(2):
                        nc.tensor.matmul(
                            scp[:, i, :w], k16[:, j * P:(j + 1) * P],
                            q16c[2 * pair + i][:, o:o + w],
                            start=True, stop=True)
                    at = att_pool.tile([P, 2, 512], BF, tag="att",
                                       name="att", bufs=3)
                    nc.scalar.activation(at[:, :, :w], scp[:, :, :w], EXP,
                                         scale=SCALE)
                    if j >= 4 * c:
                        for i in range(2):
                            nc.vector.tensor_mul(at[:, i, 0:P],
                                                 at[:, i, 0:P], tri_sb[:])
                    atts[j] = (at, o, w)

                def avdn(j, av):
                    at, o, w = atts.pop(j)
                    for i in range(2):
                        nc.tensor.matmul(
                            av[i][:, o:o + w], vtok[j], at[:, i, :w],
                            start=(j == 0), stop=(j == jmax),
                            skip_group_check=True)
                    if j == 0:
                        S2[0] = s_pool.tile([P, 2, 512], F32,
                                            tag=f"S{pair}", name="S2")
                        nc.vector.tensor_copy(S2[0][:], at[:])
                    else:
                        nc.vector.tensor_add(S2[0][:, :, o:o + w],
                                             S2[0][:, :, o:o + w],
                                             at[:, :, :w])

                av = [av_ps.tile([P, 512], F32, tag=f"av{i}",
                                 name=f"av{pair}_{i}") for i in range(2)]
                scores(0)
                for j in range(jmax + 1):
                    if j < jmax:
                        scores(j + 1)
                    if interleave is not None:
                        interleave(j)
                    avdn(j, av)
                norm_pair(c, pair, S2[0], av)

            AGK = 4   # row-blocks per o_proj operand DMA

            def load_ag_full(c):
                # ag_outF[c] rows are already kc-ordered: r*4 + 2*pair + i
                tiles = []
                for g in range(8):
                    at = ag_pool.tile([P, AGK, 512], BF,
                                      tag=f"ag{g % 2}{g // 2}", name="ag",
                                      bufs=1)
                    nc.gpsimd.dma_start(
                        at[:],
                        ag_outF[c, g * AGK * P:(g + 1) * AGK * P, :]
                        .rearrange("(k p) t -> p k t", p=P))
                    tiles.append(at)
                return tiles

            def load_ag_half(pair):
                # ag_out[pair] rows: core r at r*256, head i at +i*128
                # -> kc index for (r, i) is r*4 + 2*pair + i
                tiles = []
                for g in range(4):
                    at = ag_pool.tile([P, AGK, 512], BF,
                                      tag=f"ag{pair}{g}", name="ag", bufs=1)
                    nc.gpsimd.dma_start(
                        at[:],
                        ag_out[pair, g * AGK * P:(g + 1) * AGK * P, :]
                        .rearrange("(k p) t -> p k t", p=P))
                    tiles.append(at)
                return tiles

            def oproj_kcs(tiles):
                # yields (kc, operand AP) in PSUM-accumulation order
                for pair in range(2):
                    if tiles[pair] is None:
                        continue
                    for g in range(4):
                        for b in range(AGK):
                            r, i = (g * AGK + b) // 2, (g * AGK + b) % 2
                            kc = r * 4 + 2 * pair + i
                            yield kc, tiles[pair][g][:, b, :]



            # ================= main chunk pipeline =================
            def mm_oproj_one(c, tiles, m):
                pso = gp_ps.tile([P, 512], F32, tag="gp", name="pso",
                                 bufs=2)
                for kc in range(KCH):
                    nc.tensor.matmul(
                        pso[:], wo_sb[:, m, kc * P:(kc + 1) * P],
                        tiles[kc // AGK][:, kc % AGK, :],
                        start=(kc == 0), stop=(kc == KCH - 1))
                osb = oo_pool.tile([P, 512], F32, tag="osb",
                                   name="osb", bufs=1)
                nc.scalar.copy(osb[:], pso[:])
                nc.scalar.dma_start(
                    outT[m * P:(m + 1) * P, c * 512:(c + 1) * 512],
                    osb[:])

            for c in range(NCH):
                q16c = qkv_chunk(c)
                if c == 0:
                    for m in range(QH):
                        nc.sync.dma_start(
                            wo_sb[:, m, :], w_oT[m * P:(m + 1) * P, :])

                # o_proj(c-1): operand loads late in pair A, one m-chain
                # per j step of pair B (keeps PE fed during exp bubbles
                # without risking an in-order stall on a late AllGather)
                state = {}

                def inter_a(j, c=c, state=state):
                    if c >= 1 and j == 2:
                        state["t"] = load_ag_full(c - 1)

                def inter_b(j, c=c, state=state):
                    if c >= 1 and j < QH:
                        mm_oproj_one(c - 1, state["t"], j)

                att_pair(c, 0, q16c, interleave=inter_a)
                att_pair(c, 1, q16c, interleave=inter_b)
                if c < NCH - 1:
                    nc.gpsimd.collective_compute(
                        "AllGather", mybir.AluOpType.bypass,
                        replica_groups=[list(range(CORES))],
                        ins=[ag_in[c]], outs=[ag_outF[c]])
                if c + 2 < NCH:
                    issue_hid(c + 2)

            # tail: o_proj for chunk 3 in two phases so the A-half matmuls
            # run while AG(3,B) is still in flight
            c3 = NCH - 1
            ta = load_ag_half(0)
            oa = []
            for m in range(QH):
                psoA = gp_ps.tile([P, 512], F32, tag="gp", name="psoA",
                                  bufs=2)
                for n, (kc, ap) in enumerate(oproj_kcs((ta, None))):
                    nc.tensor.matmul(
                        psoA[:], wo_sb[:, m, kc * P:(kc + 1) * P], ap,
                        start=(n == 0), stop=(n == 15))
                oA = oo_pool.tile([P, 512], BF, tag="oA", name="oA", bufs=4)
                nc.scalar.copy(oA[:], psoA[:])
                oa.append(oA)
            tb = load_ag_half(1)
            for m in range(QH):
                psoB = gp_ps.tile([P, 512], F32, tag="gp", name="psoB",
                                  bufs=2)
                ops = list(oproj_kcs((None, tb)))
                for n, (kc, ap) in enumerate(ops):
                    nc.tensor.matmul(
                        psoB[:], wo_sb[:, m, kc * P:(kc + 1) * P], ap,
                        start=(n == 0), stop=(n == len(ops) - 1))
                osb = oo_pool.tile([P, 512], F32, tag="osb",
                                   name="osb", bufs=1)
                nc.vector.tensor_add(osb[:], psoB[:], oa[m][:])
                nc.scalar.dma_start(
                    outT[m * P:(m + 1) * P, c3 * 512:(c3 + 1) * 512],
                    osb[:])

    nc.finalize()
    return nc


_NC_CACHE = None


def _host_prep(positions, hidden_states, w_qkv, w_o):
    pos = np.asarray(positions).astype(np.float64)
    half = D // 2
    inv_freq = 1.0 / (THETA ** (np.arange(half, dtype=np.float64) * 2.0 / D))
    freqs = pos[:, None] * inv_freq[None, :]          # [T, 64]
    cos = np.cos(freqs).astype(np.float32).T          # [64, T]
    sin = np.sin(freqs).astype(np.float32).T
    # pair-interleaved basis: new row 2i <- old i, new row 2i+1 <- old 64+i
    cosT = np.empty((P, T), dtype=np.float32)
    sinT = np.empty((P, T), dtype=np.float32)
    cosT[0::2] = cos
    cosT[1::2] = cos
    sinT[0::2] = -sin
    sinT[1::2] = sin
    cosT = cosT.astype(BF_NP)
    sinT = sinT.astype(BF_NP)

    hs = np.asarray(hidden_states, dtype=np.float32)
    # hidP[c, p, kc*512+t] = hs[c*512+t, kc*128+p]
    hidP = np.ascontiguousarray(
        hs.reshape(NCH, 512, KCH, P).transpose(0, 3, 2, 1)
        .reshape(NCH, P, KCH * 512)).astype(BF_NP)

    w_qkv = np.asarray(w_qkv, dtype=np.float32)
    w_o = np.asarray(w_o, dtype=np.float32)
    # head-dim pair interleave for q and k rows (within each D block)
    perm = np.empty(D, dtype=np.int64)
    perm[0::2] = np.arange(half)
    perm[1::2] = np.arange(half) + half

    in_maps = []
    for c in range(CORES):
        k_rows = w_qkv[H * D + c * D: H * D + (c + 1) * D][perm]
        v_rows = w_qkv[(H + KV) * D + c * D: (H + KV) * D + (c + 1) * D]
        q_rows = w_qkv[c * QH * D:(c + 1) * QH * D].reshape(QH, D, HIDDEN)
        q_rows = np.ascontiguousarray(q_rows[:, perm, :]).reshape(
            QH * D, HIDDEN)
        wcat = np.concatenate([k_rows, v_rows, q_rows], axis=0)
        # stationary-major: [m*128+p, kc*128+cc] = wcat[m*128+cc, kc*128+p]
        w_qkvT_c = np.ascontiguousarray(
            wcat.reshape(SLOTS, P, KCH, P).transpose(0, 3, 2, 1)
            .reshape(SLOTS * P, KCH * P)).astype(BF_NP)
        wo_slice = w_o[c * LOCAL:(c + 1) * LOCAL, :]    # [512 out, 4096 in]
        w_oT_c = np.ascontiguousarray(
            wo_slice.reshape(QH, P, KCH, P).transpose(0, 3, 2, 1)
            .reshape(QH * P, KCH * P)).astype(BF_NP)
        in_maps.append({
            "hidP": hidP,
            "w_qkvT": w_qkvT_c,
            "w_oT": w_oT_c,
            "cosT": cosT,
            "sinT": sinT,
        })
    return in_maps


def kernel(positions, hidden_states, w_qkv, w_o):
    global _NC_CACHE
    in_maps = _host_prep(positions, hidden_states, w_qkv, w_o)
    if _NC_CACHE is None:
        _NC_CACHE = build_nc()
    res = None
    for attempt in range(3):
        try:
            res = run_bass_kernel_spmd(_NC_CACHE, in_maps,
                                       core_ids=list(range(CORES)))
            break
        except Exception:
            if attempt == 2:
                raise
    outs = [res.results[c]["outT"].T for c in range(CORES)]   # [2048, 512]
    return np.ascontiguousarray(np.concatenate(outs, axis=1))


# revision 22
# speedup vs baseline: 1.0543x; 1.0543x over previous
"""Mixtral-style GQA attention block on 8 Trainium2 NeuronCores.

Tensor-parallel over heads: core c owns q-heads [4c..4c+4) and kv-head c.
All matmuls bf16 (1 cycle/row), fp32 PSUM accumulation.

v2: chunk-pipelined single phase. For each 512-token chunk c:
  qkv(c) (6 slots x 32 kc, PE-dense) -> rope (DVE: pair-interleaved head dim
  so rotate-half is a stream_shuffle, no DMA) -> attention(c) in head-pairs
  (scores 2-at-a-time into a 2-bank PSUM group, one batched exp per j,
  softmax denominator accumulated on DVE) -> per-pair normalization ->
  AllGather(c) -> o_proj(c) slotted into the NEXT chunk's attention bubbles.
The Tile list scheduler interleaves qkv(c+1)/o_proj(c-1) matmuls into
attention's scalar-exp bubbles, keeping the PE streaming end-to-end.

DMA queues: bulk loads (weights, hid) on sync/HWDGE; latency-critical small
DMAs (v transposes, dnr rows, avn stores, out stores) on scalar/HWDGE;
AG-gated o_proj operand loads on gpsimd/SWDGE so they can't head-of-line
block either HWDGE ring. hid is host-packed partition-major per chunk so
each load is 4 big-line (8KB/partition) DMAs.

Model dims (hardcoded): T=2048, HIDDEN=4096, H=32, KV=8, D=128.
"""

import ml_dtypes
import numpy as np

import concourse.mybir as mybir
import concourse.tile as tile
from concourse import bacc
from concourse.bass_utils import run_bass_kernel_spmd

# ---- problem dims ----
T = 2048
HIDDEN = 4096
H = 32
KV = 8
D = 128
THETA = 10000.0
SCALE = D ** -0.5

CORES = 8
QH = H // CORES            # 4 q heads per core
SLOTS = QH + 2             # k, v, q0..q3 head-major slots
LOCAL = QH * D             # 512: per-core attention output dims
P = 128
NCH = T // 512             # 4 token chunks of 512
KCH = HIDDEN // P          # 32 contraction chunks
TT = T // P                # 16 token tiles of 128
GK = 8                     # kc per hid DMA group
NG = KCH // GK             # 4 hid groups per chunk

F32 = mybir.dt.float32
BF = mybir.dt.bfloat16
EXP = mybir.ActivationFunctionType.Exp
BF_NP = ml_dtypes.bfloat16

# swap adjacent partitions within each 32-lane quadrant
SWAP_MASK = [i ^ 1 for i in range(32)]


def build_nc():
    nc = bacc.Bacc(num_devices=CORES)

    # ---- per-core I/O (host pre-packs bf16 + stationary-major weights) ----
    # hidP[c, p, kc*512+t] = hidden_states[c*512+t, kc*128+p]
    hidP = nc.declare_dram_parameter("hidP", [NCH, P, KCH * 512], BF,
                                     isOutput=False)
    w_qkvT = nc.declare_dram_parameter("w_qkvT", [SLOTS * P, KCH * P], BF,
                                       isOutput=False)
    w_oT = nc.declare_dram_parameter("w_oT", [QH * P, KCH * P], BF,
                                     isOutput=False)
    cosT = nc.declare_dram_parameter("cosT", [P, T], BF, isOutput=False)
    sinT = nc.declare_dram_parameter("sinT", [P, T], BF, isOutput=False)
    outT = nc.declare_dram_parameter("outT", [LOCAL, T], F32, isOutput=True)

    # ---- consts ----
    tri_c = nc.inline_tensor(
        np.triu(np.ones((P, P), dtype=np.float32)).astype(BF_NP), name="tri_c")
    eye_c = nc.inline_tensor(
        np.eye(P, dtype=np.float32).astype(BF_NP), name="eye_c")
    ones_col_c = nc.inline_tensor(
        np.ones((P, 1), dtype=np.float32).astype(BF_NP), name="ones_col_c")
    ones_row_c = nc.inline_tensor(
        np.ones((1, P), dtype=np.float32).astype(BF_NP), name="ones_row_c")

    # ---- collective bounce buffers (chunk-major, bf16) ----
    ag_in = nc.dram_tensor("ag_in", [NCH, 2, 2 * P, 512], BF)
    ag_outF = nc.dram_tensor("ag_outF", [NCH - 1, CORES * 4 * P, 512], BF,
                             addr_space="Shared")
    ag_out = nc.dram_tensor("ag_out", [2, CORES * 2 * P, 512], BF,
                            addr_space="Shared")
    agw_in = nc.dram_tensor("agw_in", [1, 64], BF)
    agw_out = nc.dram_tensor("agw_out", [CORES, 64], BF, addr_space="Shared")

    with tile.TileContext(nc) as tc:
        with tc.tile_pool(name="const", bufs=1) as cpool, \
             tc.tile_pool(name="wq", bufs=1) as wq_pool, \
             tc.tile_pool(name="wo", bufs=1) as wo_pool, \
             tc.tile_pool(name="cs", bufs=1) as cs_pool, \
             tc.tile_pool(name="hid", bufs=1) as hid_pool, \
             tc.tile_pool(name="qk", bufs=1) as qk_pool, \
             tc.tile_pool(name="att", bufs=1) as att_pool, \
             tc.tile_pool(name="ssum", bufs=1) as s_pool, \
             tc.tile_pool(name="sm", bufs=1) as sm_pool, \
             tc.tile_pool(name="rp", bufs=1) as rp_pool, \
             tc.tile_pool(name="agp", bufs=1) as ag_pool, \
             tc.tile_pool(name="oo", bufs=1) as oo_pool, \
             tc.tile_pool(name="scp", bufs=1, space="PSUM") as scp_ps, \
             tc.tile_pool(name="avp", bufs=1, space="PSUM") as av_ps, \
             tc.tile_pool(name="gp", bufs=1, space="PSUM") as gp_ps:

            # ---- initial bulk DMAs (sync/HWDGE queue, consumption order) --
            # warm up the collective path before anything else on PL
            nc.gpsimd.collective_compute(
                "AllGather", mybir.AluOpType.bypass,
                replica_groups=[list(range(CORES))],
                ins=[agw_in[:]], outs=[agw_out[:]])

            wms = [wq_pool.tile([P, KCH * P], BF, tag=f"wm{m}",
                                name=f"wm{m}") for m in range(SLOTS)]

            def issue_wm(m, parts=2, eng=None):
                eng = eng or nc.sync
                step = KCH * P // parts
                for s in range(parts):
                    eng.dma_start(wms[m][:, s * step:(s + 1) * step],
                                  w_qkvT[m * P:(m + 1) * P,
                                         s * step:(s + 1) * step])

            hts = {}     # (chunk, group) -> tile

            def issue_hid(c, g0=0, halves=False):
                for g in range(g0, NG):
                    ht = hid_pool.tile([P, GK * 512], BF, tag=f"ht{g}",
                                       name=f"ht{c}_{g}", bufs=1)
                    if halves:
                        hw = GK * 512 // 2
                        nc.sync.dma_start(
                            ht[:, :hw], hidP[c, :, g * GK * 512:
                                             g * GK * 512 + hw])
                        nc.sync.dma_start(
                            ht[:, hw:], hidP[c, :, g * GK * 512 + hw:
                                             (g + 1) * GK * 512])
                    else:
                        nc.sync.dma_start(
                            ht[:], hidP[c, :,
                                        g * GK * 512:(g + 1) * GK * 512])
                    hts[(c, g)] = ht

            issue_wm(0, parts=4)
            issue_hid(0, g0=0, halves=True)
            issue_wm(1)
            cos_sb = cs_pool.tile([P, T], BF, tag="cos")
            sin_sb = cs_pool.tile([P, T], BF, tag="sin")
            nc.sync.dma_start(cos_sb[:], cosT[:, :])
            nc.sync.dma_start(sin_sb[:], sinT[:, :])
            tri_sb = cpool.tile([P, P], BF, tag="tri")
            nc.sync.dma_start(tri_sb[:], tri_c[:, :])
            onc1_sb = cpool.tile([P, 1], BF, tag="onc1")
            nc.sync.dma_start(onc1_sb[:], ones_col_c[:, :])
            onr_sb = cpool.tile([1, P], BF, tag="onr")
            nc.sync.dma_start(onr_sb[:], ones_row_c[:, :])
            eye_sb = cpool.tile([P, P], BF, tag="eye")
            nc.sync.dma_start(eye_sb[:], eye_c[:, :])
            for m in range(2, SLOTS):
                issue_wm(m, eng=nc.gpsimd)
            wo_sb = wo_pool.tile([P, QH, KCH * P], BF, tag="wo")

            # ---- persistent attention operands ----
            k16 = qk_pool.tile([P, T], BF, tag="k", name="k16")
            vtok = {}

            def rope_to(dst, psq, c):
                """dst[:, :512] = rope(psq) in the pair-interleaved basis."""
                t0 = c * 512
                xs = rp_pool.tile([P, 512], BF, tag="xs", name="xs", bufs=2)
                nc.scalar.copy(xs[:], psq[:])
                xr = rp_pool.tile([P, 512], BF, tag="xr", name="xr", bufs=2)
                nc.vector.stream_shuffle(xr[:], xs[:], mask=SWAP_MASK)
                tcs = rp_pool.tile([P, 512], BF, tag="tc", name="tc", bufs=2)
                nc.vector.tensor_mul(tcs[:], xs[:], cos_sb[:, t0:t0 + 512])
                tsn = rp_pool.tile([P, 512], BF, tag="ts", name="ts", bufs=1)
                nc.vector.tensor_mul(tsn[:], xr[:], sin_sb[:, t0:t0 + 512])
                nc.vector.tensor_add(dst, tcs[:], tsn[:])

            def qkv_chunk(c):
                q16c = []
                for m in range(SLOTS):
                    psq = gp_ps.tile([P, 512], F32, tag="gp", name="psq",
                                     bufs=2)
                    for kc in range(KCH):
                        g, o = kc // GK, (kc % GK) * 512
                        nc.tensor.matmul(
                            psq[:], wms[m][:, kc * P:(kc + 1) * P],
                            hts[(c, g)][:, o:o + 512],
                            start=(kc == 0), stop=(kc == KCH - 1))
                    if m == 0:
                        rope_to(k16[:, c * 512:(c + 1) * 512], psq, c)
                    elif m == 1:
                        vtmp = rp_pool.tile([P, 512], BF, tag="vtmp",
                                            name="vtmp", bufs=1)
                        nc.scalar.copy(vtmp[:], psq[:])
                        vt_ps = gp_ps.tile([P, 4 * P], BF, tag="gp",
                                           name="vt_ps", bufs=2)
                        for jj in range(4):
                            nc.tensor.transpose(
                                vt_ps[:, jj * P:(jj + 1) * P],
                                vtmp[:, jj * P:(jj + 1) * P], eye_sb[:])
                        vt_c = qk_pool.tile([P, 4 * P], BF, tag=f"vtc{c}",
                                            name=f"vtc{c}")
                        nc.vector.tensor_copy(vt_c[:], vt_ps[:])
                        for jj in range(4):
                            vtok[c * 4 + jj] = vt_c[:, jj * P:(jj + 1) * P]
                        if c == 0:
                            issue_hid(1)
                    else:
                        qt = qk_pool.tile([P, 512], BF, tag=f"q{m - 2}",
                                          name=f"q{m - 2}_{c}", bufs=2)
                        rope_to(qt[:], psq, c)
                        q16c.append(qt)
                return q16c

            def norm_pair(c, pair, S2, av):
                """softmax-normalize the pair's AV psum, store to ag_in."""
                s16 = sm_pool.tile([P, 2, 512], BF, tag="s16", name="s16",
                                   bufs=1)
                nc.vector.tensor_copy(s16[:], S2[:])
                # both heads' denominators as one partition-0 row [1, 1024]
                dnp = gp_ps.tile([P, 512], F32, tag="gp", name="dnp", bufs=2)
                dnp2 = gp_ps.tile([P, 512], F32, tag="gp", name="dnp2",
                                  bufs=2)
                nc.tensor.matmul(dnp[0:1, :], onc1_sb[:], s16[:, 0, :],
                                 start=True, stop=True)
                nc.tensor.matmul(dnp2[0:1, :], onc1_sb[:], s16[:, 1, :],
                                 start=True, stop=True)
                rc = sm_pool.tile([1, 2, 512], F32, tag="rc", bufs=1)
                nc.vector.reciprocal_approx_fast(rc[:, 0, :], dnp[0:1, :])
                nc.vector.reciprocal_approx_fast(rc[:, 1, :], dnp2[0:1, :])
                rc16 = sm_pool.tile([1, 2, 512], BF, tag="rc16", bufs=1)
                nc.scalar.copy(rc16[:], rc[:])
                for i in range(2):
                    bcp = gp_ps.tile([P, 512], F32, tag="gp", name="bcp",
                                     bufs=2)
                    nc.tensor.matmul(bcp[:], onr_sb[:], rc16[:, i, :],
                                     start=True, stop=True)
                    bc_sb = sm_pool.tile([P, 512], F32, tag="bc_sb", bufs=1)
                    nc.scalar.copy(bc_sb[:], bcp[:])
                    avn = sm_pool.tile([P, 512], BF, tag="avn", bufs=1)
                    nc.vector.tensor_mul(avn[:], av[i][:], bc_sb[:])
                    nc.scalar.dma_start(ag_in[c, pair, i * P:(i + 1) * P, :],
                                        avn[:])
                if c == NCH - 1:
                    nc.gpsimd.collective_compute(
                        "AllGather", mybir.AluOpType.bypass,
                        replica_groups=[list(range(CORES))],
                        ins=[ag_in[c, pair]], outs=[ag_out[pair]])

            def att_pair(c, pair, q16c, interleave=None):
                t0 = c * 512
                jmax = 4 * c + 3
                S2 = [None]
                atts = {}

                def scores(j):
                    toff = max(t0, j * P)
                    w = t0 + 512 - toff
                    o = toff - t0
                    scp = scp_ps.tile([P, 2, 512], F32, tag="scp",
                                      name="scp", bufs=2)
                    for i in range(2):
                        nc.tensor.matmul(
                            scp[:, i, :w], k16[:, j * P:(j + 1) * P],
                            q16c[2 * pair + i][:, o:o + w],
                            start=True, stop=True)
                    at = att_pool.tile([P, 2, 512], BF, tag="att",
                                       name="att", bufs=3)
                    nc.scalar.activation(at[:, :, :w], scp[:, :, :w], EXP,
                                         scale=SCALE)
                    if j >= 4 * c:
                        for i in range(2):
                            nc.vector.tensor_mul(at[:, i, 0:P],
                                                 at[:, i, 0:P], tri_sb[:])
                    atts[j] = (at, o, w)

                def avdn(j, av):
                    at, o, w = atts.pop(j)
                    for i in range(2):
                        nc.tensor.matmul(
                            av[i][:, o:o + w], vtok[j], at[:, i, :w],
                            start=(j == 0), stop=(j == jmax),
                            skip_group_check=True)
                    if j == 0:
                        S2[0] = s_pool.tile([P, 2, 512], F32,
                                            tag=f"S{pair}", name="S2")
                        nc.vector.tensor_copy(S2[0][:], at[:])
                    else:
                        nc.vector.tensor_add(S2[0][:, :, o:o + w],
                                             S2[0][:, :, o:o + w],
                                             at[:, :, :w])

                av = [av_ps.tile([P, 512], F32, tag=f"av{i}",
                                 name=f"av{pair}_{i}") for i in range(2)]
                scores(0)
                for j in range(jmax + 1):
                    if j < jmax:
                        scores(j + 1)
                    if interleave is not None:
                        interleave(j)
                    avdn(j, av)
                norm_pair(c, pair, S2[0], av)

            AGK = 4   # row-blocks per o_proj operand DMA

            def load_ag_full(c):
                # ag_outF[c] rows are already kc-ordered: r*4 + 2*pair + i
                tiles = []
                for g in range(8):
                    at = ag_pool.tile([P, AGK, 512], BF,
                                      tag=f"ag{g % 2}{g // 2}", name="ag",
                                      bufs=1)
                    nc.gpsimd.dma_start(
                        at[:],
                        ag_outF[c, g * AGK * P:(g + 1) * AGK * P, :]
                        .rearrange("(k p) t -> p k t", p=P))
                    tiles.append(at)
                return tiles

            def load_ag_half(pair):
                # ag_out[pair] rows: core r at r*256, head i at +i*128
                # -> kc index for (r, i) is r*4 + 2*pair + i
                tiles = []
                for g in range(4):
                    at = ag_pool.tile([P, AGK, 512], BF,
                                      tag=f"ag{pair}{g}", name="ag", bufs=1)
                    nc.gpsimd.dma_start(
                        at[:],
                        ag_out[pair, g * AGK * P:(g + 1) * AGK * P, :]
                        .rearrange("(k p) t -> p k t", p=P))
                    tiles.append(at)
                return tiles

            def oproj_kcs(tiles):
                # yields (kc, operand AP) in PSUM-accumulation order
                for pair in range(2):
                    if tiles[pair] is None:
                        continue
                    for g in range(4):
                        for b in range(AGK):
                            r, i = (g * AGK + b) // 2, (g * AGK + b) % 2
                            kc = r * 4 + 2 * pair + i
                            yield kc, tiles[pair][g][:, b, :]



            # ================= main chunk pipeline =================
            def mm_oproj_one(c, tiles, m):
                pso = gp_ps.tile([P, 512], F32, tag="gp", name="pso",
                                 bufs=2)
                for kc in range(KCH):
                    nc.tensor.matmul(
                        pso[:], wo_sb[:, m, kc * P:(kc + 1) * P],
                        tiles[kc // AGK][:, kc % AGK, :],
                        start=(kc == 0), stop=(kc == KCH - 1))
                osb = oo_pool.tile([P, 512], F32, tag="osb",
                                   name="osb", bufs=1)
                nc.scalar.copy(osb[:], pso[:])
                nc.scalar.dma_start(
                    outT[m * P:(m + 1) * P, c * 512:(c + 1) * 512],
                    osb[:])

            for c in range(NCH):
                q16c = qkv_chunk(c)
                if c == 0:
                    for m in range(QH):
                        nc.sync.dma_start(
                            wo_sb[:, m, :], w_oT[m * P:(m + 1) * P, :])

                # o_proj(c-1): operand loads late in pair A, one m-chain
                # per j step of pair B (keeps PE fed during exp bubbles
                # without risking an in-order stall on a late AllGather)
                state = {}

                def inter_a(j, c=c, state=state):
                    if c >= 1 and j == 2:
                        state["t"] = load_ag_full(c - 1)

                def inter_b(j, c=c, state=state):
                    if c >= 1 and j < QH:
                        mm_oproj_one(c - 1, state["t"], j)
                    if c == NCH - 1 and j == QH:
                        state["ta3"] = load_ag_half(0)

                att_pair(c, 0, q16c, interleave=inter_a)
                att_pair(c, 1, q16c, interleave=inter_b)
                if c < NCH - 1:
                    nc.gpsimd.collective_compute(
                        "AllGather", mybir.AluOpType.bypass,
                        replica_groups=[list(range(CORES))],
                        ins=[ag_in[c]], outs=[ag_outF[c]])
                if c + 2 < NCH:
                    issue_hid(c + 2)

            # tail: o_proj for chunk 3 in two phases so the A-half matmuls
            # run while AG(3,B) is still in flight (A-half operand loads were
            # issued inside att(3) pair B, ahead of AG(3,B) on the PL queue)
            c3 = NCH - 1
            ta = state["ta3"]
            oa = []
            for m in range(QH):
                psoA = gp_ps.tile([P, 512], F32, tag="gp", name="psoA",
                                  bufs=2)
                for n, (kc, ap) in enumerate(oproj_kcs((ta, None))):
                    nc.tensor.matmul(
                        psoA[:], wo_sb[:, m, kc * P:(kc + 1) * P], ap,
                        start=(n == 0), stop=(n == 15))
                oA = oo_pool.tile([P, 512], BF, tag="oA", name="oA", bufs=4)
                nc.scalar.copy(oA[:], psoA[:])
                oa.append(oA)
            tb = load_ag_half(1)
            for m in range(QH):
                psoB = gp_ps.tile([P, 512], F32, tag="gp", name="psoB",
                                  bufs=2)
                ops = list(oproj_kcs((None, tb)))
                for n, (kc, ap) in enumerate(ops):
                    nc.tensor.matmul(
                        psoB[:], wo_sb[:, m, kc * P:(kc + 1) * P], ap,
                        start=(n == 0), stop=(n == len(ops) - 1))
                osb = oo_pool.tile([P, 512], F32, tag="osb",
                                   name="osb", bufs=1)
                nc.vector.tensor_add(osb[:], psoB[:], oa[m][:])
                nc.scalar.dma_start(
                    outT[m * P:(m + 1) * P, c3 * 512:(c3 + 1) * 512],
                    osb[:])

    nc.finalize()
    return nc


_NC_CACHE = None


def _host_prep(positions, hidden_states, w_qkv, w_o):
    pos = np.asarray(positions).astype(np.float64)
    half = D // 2
    inv_freq = 1.0 / (THETA ** (np.arange(half, dtype=np.float64) * 2.0 / D))
    freqs = pos[:, None] * inv_freq[None, :]          # [T, 64]
    cos = np.cos(freqs).astype(np.float32).T          # [64, T]
    sin = np.sin(freqs).astype(np.float32).T
    # pair-interleaved basis: new row 2i <- old i, new row 2i+1 <- old 64+i
    cosT = np.empty((P, T), dtype=np.float32)
    sinT = np.empty((P, T), dtype=np.float32)
    cosT[0::2] = cos
    cosT[1::2] = cos
    sinT[0::2] = -sin
    sinT[1::2] = sin
    cosT = cosT.astype(BF_NP)
    sinT = sinT.astype(BF_NP)

    hs = np.asarray(hidden_states, dtype=np.float32)
    # hidP[c, p, kc*512+t] = hs[c*512+t, kc*128+p]
    hidP = np.ascontiguousarray(
        hs.reshape(NCH, 512, KCH, P).transpose(0, 3, 2, 1)
        .reshape(NCH, P, KCH * 512)).astype(BF_NP)

    w_qkv = np.asarray(w_qkv, dtype=np.float32)
    w_o = np.asarray(w_o, dtype=np.float32)
    # head-dim pair interleave for q and k rows (within each D block)
    perm = np.empty(D, dtype=np.int64)
    perm[0::2] = np.arange(half)
    perm[1::2] = np.arange(half) + half

    in_maps = []
    for c in range(CORES):
        k_rows = w_qkv[H * D + c * D: H * D + (c + 1) * D][perm]
        v_rows = w_qkv[(H + KV) * D + c * D: (H + KV) * D + (c + 1) * D]
        q_rows = w_qkv[c * QH * D:(c + 1) * QH * D].reshape(QH, D, HIDDEN)
        q_rows = np.ascontiguousarray(q_rows[:, perm, :]).reshape(
            QH * D, HIDDEN)
        wcat = np.concatenate([k_rows, v_rows, q_rows], axis=0)
        # stationary-major: [m*128+p, kc*128+cc] = wcat[m*128+cc, kc*128+p]
        w_qkvT_c = np.ascontiguousarray(
            wcat.reshape(SLOTS, P, KCH, P).transpose(0, 3, 2, 1)
            .reshape(SLOTS * P, KCH * P)).astype(BF_NP)
        wo_slice = w_o[c * LOCAL:(c + 1) * LOCAL, :]    # [512 out, 4096 in]
        w_oT_c = np.ascontiguousarray(
            wo_slice.reshape(QH, P, KCH, P).transpose(0, 3, 2, 1)
            .reshape(QH * P, KCH * P)).astype(BF_NP)
        in_maps.append({
            "hidP": hidP,
            "w_qkvT": w_qkvT_c,
            "w_oT": w_oT_c,
            "cosT": cosT,
            "sinT": sinT,
        })
    return in_maps


def kernel(positions, hidden_states, w_qkv, w_o):
    global _NC_CACHE
    in_maps = _host_prep(positions, hidden_states, w_qkv, w_o)
    if _NC_CACHE is None:
        _NC_CACHE = build_nc()
    res = None
    for attempt in range(3):
        try:
            res = run_bass_kernel_spmd(_NC_CACHE, in_maps,
                                       core_ids=list(range(CORES)))
            break
        except Exception:
            if attempt == 2:
                raise
    outs = [res.results[c]["outT"].T for c in range(CORES)]   # [2048, 512]
    return np.ascontiguousarray(np.concatenate(outs, axis=1))


# revision 26
# speedup vs baseline: 1.1389x; 1.0802x over previous
"""Mixtral-style GQA attention block on 8 Trainium2 NeuronCores.

Tensor-parallel over heads: core c owns q-heads [4c..4c+4) and kv-head c.
All matmuls run in bf16 (1 cycle/row on the PE vs fp32r's 2-pass mode);
accumulation stays fp32 in PSUM. Numerics check: end-to-end bf16 gives
rel_l2 ~8e-3 vs the fp32 reference (gate is 2e-2).

Pipeline per core:
  qkv proj (bf16, full-32-chunk PSUM accumulation; k/v slots fused into
  one hid-arrival-paced loop to hide the 16MB activation load)
  -> RoPE (rotate-half via SBUF partition-swap DMA + DVE muls, no PE)
  -> causal attention in token-chunk order 1,2,3,0 (transposed-scores
     layout; softmax denominator accumulated on the DVE; per-chunk bf16
     AllGather triggered two pipeline steps after each chunk)
  -> o_proj per chunk at the tail; the list scheduler slots its matmuls
     into attention's scalar-exp bubbles, and the tail chain hangs off
     the smallest chunk so the last AllGather hides under o_proj work.
ag tiles share the attention tile pool so their AG-blocked DMAs can't
head-of-line-block the in-order sync engine ahead of latency-critical
norm DMAs. Host concatenates the per-core column slices.

Model dims (hardcoded): T=2048, HIDDEN=4096, H=32, KV=8, D=128.
"""

from contextlib import ExitStack

import ml_dtypes
import numpy as np

import concourse.bass_utils as _bu
import concourse.mybir as mybir
import concourse.tile as tile
from concourse import bacc
from concourse.bass_utils import run_bass_kernel_spmd

# ---- problem dims ----
T = 2048
HIDDEN = 4096
H = 32
KV = 8
D = 128
THETA = 10000.0
SCALE = D ** -0.5

CORES = 8
QH = H // CORES            # 4 q heads per core
SLOTS = QH + 2             # k, v, q0..q3 head-major slots
LOCAL = QH * D             # 512: per-core attention output dims
P = 128
NCH = T // 512             # 4 token chunks of 512
KCH = HIDDEN // P          # 32 contraction chunks
TT = T // P                # 16 token tiles of 128

F32 = mybir.dt.float32
F32R = mybir.dt.float32r
BF = mybir.dt.bfloat16
SWAP_MASK = [i ^ 1 for i in range(32)]
EXP = mybir.ActivationFunctionType.Exp
BF_NP = ml_dtypes.bfloat16


def build_nc():
    nc = bacc.Bacc(num_devices=CORES)

    # ---- per-core I/O (host pre-packs bf16 + stationary-major weights) ----
    hidT = nc.declare_dram_parameter("hidT", [HIDDEN, T], BF, isOutput=False)
    # w_qkvT[m*128+p, kc*128+c] = W_slot_m[c-th out row, kc*128+p]
    w_qkvT = nc.declare_dram_parameter("w_qkvT", [SLOTS * P, KCH * P], BF,
                                       isOutput=False)
    w_oT = nc.declare_dram_parameter("w_oT", [QH * P, KCH * P], BF,
                                     isOutput=False)
    cosT = nc.declare_dram_parameter("cosT", [P, T], BF, isOutput=False)
    sinT = nc.declare_dram_parameter("sinT", [P, T], BF, isOutput=False)
    outT = nc.declare_dram_parameter("outT", [LOCAL, T], F32, isOutput=True)

    # ---- consts ----
    tri_c = nc.inline_tensor(
        np.triu(np.ones((P, P), dtype=np.float32)).astype(BF_NP), name="tri_c")
    # dn lhsT for head h: [128, 4] with column h all-ones
    onc4 = np.zeros((P, QH, QH), dtype=np.float32)
    for h in range(QH):
        onc4[:, h, h] = 1.0
    onc4_c = nc.inline_tensor(
        np.ascontiguousarray(onc4.transpose(1, 0, 2)).astype(BF_NP),
        name="onc4_c")   # [QH, 128, 4]
    ones_row_c = nc.inline_tensor(
        np.ones((1, P), dtype=np.float32).astype(BF_NP), name="ones_row_c")
    eye_c = nc.inline_tensor(
        np.eye(P, dtype=np.float32).astype(BF_NP), name="eye_c")

    # ---- collective bounce buffers (chunk-major, bf16) ----
    ag_in = nc.dram_tensor("ag_in", [NCH, LOCAL, 512], BF)
    ag_out = nc.dram_tensor("ag_out", [NCH, H * D, 512], BF,
                            addr_space="Shared")
    # tiny warmup collective: absorbs CC cold-start before AG(0)
    agw_in = nc.dram_tensor("agw_in", [1, 64], BF)
    agw_out = nc.dram_tensor("agw_out", [CORES, 64], BF, addr_space="Shared")
    # chunk 0 (the tail chunk) gathers in two 256-col halves so its o_proj
    # can start after the first half-flight
    agh_in = [nc.dram_tensor(f"agh_in{i}", [LOCAL, 256], BF) for i in (0, 1)]
    agh_out = [nc.dram_tensor(f"agh_out{i}", [H * D, 256], BF,
                              addr_space="Shared") for i in (0, 1)]

    with tile.TileContext(nc) as tc:
        with tc.tile_pool(name="const", bufs=1) as cpool:
            pstack = ExitStack()
            qpool = pstack.enter_context(tc.tile_pool(name="qk_out", bufs=1))

            # persistent bf16 attention operands
            q16 = [qpool.tile([P, T], BF, tag=f"q{h}", name=f"q{h}")
                   for h in range(QH)]
            k16 = qpool.tile([P, T], BF, tag="k", name="k")
            vtok = [qpool.tile([P, P], BF, tag=f"vt{j}", name=f"vt{j}")
                    for j in range(TT)]

            tri_sb = cpool.tile([P, P], BF, tag="tri")
            onc4_sb = [cpool.tile([P, QH], BF, tag=f"onc4_{h}",
                                  name=f"onc4_{h}") for h in range(QH)]
            onr_sb = cpool.tile([1, P], BF, tag="onr")

            # ============ phase 1: qkv proj + fused rope ====
            ph1 = ExitStack()
            hid_pool = ph1.enter_context(tc.tile_pool(name="hid", bufs=1))
            wq_pool = ph1.enter_context(tc.tile_pool(name="wq", bufs=1))
            cs_pool = ph1.enter_context(tc.tile_pool(name="cs", bufs=1))
            rp_pool = ph1.enter_context(tc.tile_pool(name="rp", bufs=1))
            pr_ps = ph1.enter_context(
                tc.tile_pool(name="pr_ps", bufs=1, space="PSUM"))

            # warm up the collective path before anything else on PL
            nc.gpsimd.collective_compute(
                "AllGather",
                mybir.AluOpType.bypass,
                replica_groups=[list(range(CORES))],
                ins=[agw_in[:]],
                outs=[agw_out[:]],
            )
            # DMA issue order matters: the first matmul needs wm(m=0) and
            # ht[0], so those go first; everything else follows.  Slots 2..5
            # go through the idle SWDGE (gpsimd) queue so they are not stuck
            # behind the 16MB hid stream on the sync ring.
            wms = []
            for m in range(SLOTS):
                wm = wq_pool.tile([P, KCH * P], BF, tag="wm",
                                  name="wm", bufs=3)
                if m < 2:
                    nc.sync.dma_start(wm[:], w_qkvT[m * P:(m + 1) * P, :])
                else:
                    nc.gpsimd.dma_start(wm[:], w_qkvT[m * P:(m + 1) * P, :])
                wms.append(wm)
            hts = []
            for kc in range(KCH):
                ht = hid_pool.tile([P, T], BF, tag=f"hid{kc}", name="ht")
                nc.sync.dma_start(ht[:], hidT[kc * P:(kc + 1) * P, :])
                hts.append(ht)
            cos_sb = cs_pool.tile([P, T], BF, tag="cos")
            sin_sb = cs_pool.tile([P, T], BF, tag="sin")
            nc.sync.dma_start(cos_sb[:], cosT[:, :])
            nc.sync.dma_start(sin_sb[:], sinT[:, :])
            nc.sync.dma_start(tri_sb[:], tri_c[:, :])
            eye_sb = cpool.tile([P, P], BF, tag="eye")
            nc.sync.dma_start(eye_sb[:], eye_c[:, :])
            for h in range(QH):
                nc.sync.dma_start(onc4_sb[h][:], onc4_c[h])
            nc.sync.dma_start(onr_sb[:], ones_row_c[:, :])

            def rope_chunk(dst16, n, ps):
                """dst16[:, chunk n] = x*cos + rot(x)*sin.

                The head dim is pair-interleaved on the host, so rotate-half
                is a swap of adjacent partitions: a DVE stream_shuffle."""
                t0 = n * 512
                xs = rp_pool.tile([P, 512], BF, tag="xs", name="xs", bufs=3)
                nc.scalar.copy(xs[:], ps[:])
                xr = rp_pool.tile([P, 512], BF, tag="xr", name="xr", bufs=3)
                nc.vector.stream_shuffle(xr[:], xs[:], mask=SWAP_MASK)
                tcos = rp_pool.tile([P, 512], BF, tag="tc", name="tc", bufs=3)
                nc.vector.tensor_mul(tcos[:], xs[:], cos_sb[:, t0:t0 + 512])
                tsin = rp_pool.tile([P, 512], BF, tag="ts", name="ts", bufs=3)
                nc.vector.tensor_mul(tsin[:], xr[:], sin_sb[:, t0:t0 + 512])
                nc.vector.tensor_add(dst16[:, t0:t0 + 512], tcos[:], tsin[:])

            # slot order: k, v, q0..q3 (host packs weights accordingly).
            # m=0 (k) and m=1 (v) run in one hid-arrival-paced kc loop so the
            # PE has 2x work per arriving hid tile while the 16MB hid load
            # streams in; m=2..5 then run at full speed on resident tiles.
            ps_k = [pr_ps.tile([P, 512], F32, tag=f"pp0_{n}", name="pp")
                    for n in range(NCH)]
            ps_v = [pr_ps.tile([P, 512], F32, tag=f"pp1_{n}", name="pp")
                    for n in range(NCH)]
            for kc in range(KCH):
                for n in range(NCH):
                    nc.tensor.matmul(
                        ps_k[n][:], wms[0][:, kc * P:(kc + 1) * P],
                        hts[kc][:, n * 512:(n + 1) * 512],
                        start=(kc == 0), stop=(kc == KCH - 1))
                for n in range(NCH):
                    nc.tensor.matmul(
                        ps_v[n][:], wms[1][:, kc * P:(kc + 1) * P],
                        hts[kc][:, n * 512:(n + 1) * 512],
                        start=(kc == 0), stop=(kc == KCH - 1))
            for n in range(NCH):
                rope_chunk(k16, n, ps_k[n])
            for n in range(NCH):
                vtmp = rp_pool.tile([P, 512], BF, tag="vtmp",
                                    name="vtmp", bufs=2)
                nc.scalar.copy(vtmp[:], ps_v[n][:])
                vt_ps = pr_ps.tile([P, 1024], BF, tag=f"pp1_{n}",
                                   name="vt_ps")
                for jj in range(4):
                    nc.tensor.transpose(
                        vt_ps[:, jj * P:(jj + 1) * P],
                        vtmp[:, jj * P:(jj + 1) * P], eye_sb[:])
                for jj in range(4):
                    j = n * 4 + jj
                    nc.vector.tensor_copy(
                        vtok[j][:], vt_ps[:, jj * P:(jj + 1) * P])
            for m in range(2, SLOTS):
                wm = wms[m]
                ps = [pr_ps.tile([P, 512], F32, tag=f"pp{m % 2}_{n}",
                                 name="pp") for n in range(NCH)]
                for kc in range(KCH):
                    for n in range(NCH):
                        nc.tensor.matmul(
                            ps[n][:], wm[:, kc * P:(kc + 1) * P],
                            hts[kc][:, n * 512:(n + 1) * 512],
                            start=(kc == 0), stop=(kc == KCH - 1))
                for n in range(NCH):
                    rope_chunk(q16[m - 2], n, ps[n])

            ph1.close()   # free hid/wq/cos/rope SBUF + qkv PSUM banks

            # ============ phase 2: attention + AG; o_proj after ====
            with tc.tile_pool(name="att", bufs=1) as att_pool, \
                 tc.tile_pool(name="ps2", bufs=1, space="PSUM") as ps2, \
                 tc.tile_pool(name="sm", bufs=2) as sm_pool, \
                 tc.tile_pool(name="ssum", bufs=1) as s_pool, \
                  tc.tile_pool(name="wo", bufs=1) as wo_pool, \
                 tc.tile_pool(name="oo", bufs=1) as oo_pool:

                wo_sb = wo_pool.tile([P, QH, KCH * P], BF, tag="wo")
                nc.sync.dma_start(
                    wo_sb[:], w_oT[:, :].rearrange("(m p) f -> p m f", p=P))

                # ag tiles share the att pool: the rotation WAR dependency
                # stops the scheduler from hoisting these sync-engine DMAs
                # (which block on AG completion) ahead of latency-critical
                # norm DMAs — head-of-line blocking on the in-order sync
                # engine cost 40us otherwise.
                def load_ag(c):
                    tiles = []
                    for kc in range(KCH):
                        at = att_pool.tile([P, 512], BF, tag="att", name="ag",
                                           bufs=88)
                        nc.sync.dma_start(
                            at[:], ag_out[c, kc * P:(kc + 1) * P, :])
                        tiles.append(at)
                    return tiles

                def load_ag_half(i):
                    tiles = []
                    for kc in range(KCH):
                        at = att_pool.tile([P, 512], BF, tag="att", name="agh",
                                           bufs=88)
                        nc.sync.dma_start(
                            at[:, :256], agh_out[i][kc * P:(kc + 1) * P, :])
                        tiles.append(at)
                    return tiles

                def mm_oproj(c, tiles, ms=range(QH), w=512, coff=0):
                    for m in ms:
                        pso = ps2.tile([P, 512], F32, tag="op", name="op",
                                       bufs=2)
                        for kc in range(KCH):
                            nc.tensor.matmul(
                                pso[:, :w], wo_sb[:, m, kc * P:(kc + 1) * P],
                                tiles[kc][:, :w],
                                start=(kc == 0), stop=(kc == KCH - 1))
                        osb = oo_pool.tile([P, 512], F32, tag="osb",
                                           name="osb", bufs=3)
                        nc.scalar.copy(osb[:, :w], pso[:, :w])
                        nc.sync.dma_start(
                            outT[m * P:(m + 1) * P,
                                 c * 512 + coff:c * 512 + coff + w],
                            osb[:, :w])

                # chunk order 1,2,3,0: the tail chain (last norm -> last AG
                # -> last o_proj) hangs off the TINY chunk 0 instead of the
                # big scalar-exp-paced chunk 3, and the big chunks' AGs and
                # o_proj overlap mid-run attention
                norm_pending = None
                ag_tiles = {}
                for idx, c in enumerate([1, 2, 3, 0]):
                    t0 = c * 512
                    jmax = 4 * c + 3
                    avp = [ps2.tile([P, 512], F32, tag=f"av{h}",
                                    name=f"av{h}") for h in range(QH)]
                    # softmax denominator accumulators (DVE, fp32)
                    S = [s_pool.tile([P, 512], F32, tag=f"s{h}",
                                     name=f"s{h}", bufs=2)
                         for h in range(QH)]
                    atts = {}

                    def scores(j, c=c, t0=t0, atts=atts):
                        toff = max(t0, j * P)
                        w = t0 + 512 - toff
                        for h in range(QH):
                            scp = ps2.tile([P, 512], F32, tag="sc",
                                           name="scp", bufs=2)
                            nc.tensor.matmul(
                                scp[:, :w], k16[:, j * P:(j + 1) * P],
                                q16[h][:, toff:toff + w],
                                start=True, stop=True)
                            att = att_pool.tile([P, 512], BF, tag="att",
                                                name="att", bufs=88)
                            nc.scalar.activation(att[:, :w], scp[:, :w], EXP,
                                                 scale=SCALE)
                            if j >= 4 * c:  # diagonal block: causal mask
                                nc.vector.tensor_mul(att[:, :P], att[:, :P],
                                                     tri_sb[:])
                            atts[(j, h)] = (att, toff, w)

                    def avdn(j, c=c, t0=t0, jmax=jmax, atts=atts, avp=avp,
                             S=S):
                        for h in range(QH):
                            att, toff, w = atts[(j, h)]
                            o = toff - t0
                            nc.tensor.matmul(
                                avp[h][:, o:o + w], vtok[j][:], att[:, :w],
                                start=(j == 0), stop=(j == jmax),
                                skip_group_check=True)
                        for h in range(QH):
                            att, toff, w = atts[(j, h)]
                            o = toff - t0
                            if j == 0:
                                nc.vector.tensor_copy(S[h][:], att[:])
                            else:
                                nc.vector.tensor_add(
                                    S[h][:, o:o + w], S[h][:, o:o + w],
                                    att[:, :w])

                    def make_norm(c=c, avp=avp, S=S):
                        dnrs = []
                        # issued at chunk end: s16 casts (DVE) feed the dn
                        # matmul without waiting; avp psum->SBUF copies
                        # (scalar) free the av banks so the next chunk's
                        # first AV matmul never waits on this chunk's norm
                        s16h = []
                        for h in range(QH):
                            s16 = sm_pool.tile([P, 512], BF, tag="s16",
                                               name="s16", bufs=4)
                            nc.vector.tensor_copy(s16[:], S[h][:])
                            s16h.append(s16)
                        avcp = []
                        for h in range(QH):
                            av_sb = sm_pool.tile([P, 512], F32, tag="avcp",
                                                 name="avcp", bufs=4)
                            nc.scalar.copy(av_sb[:], avp[h][:])
                            avcp.append(av_sb)

                        def norm_a():
                            # reciprocal-of-denominator pipeline head: ends
                            # in the dnr row DMAs so the PE-side bcp (in
                            # norm_b, two j-iterations later) never waits on
                            # the DMA roundtrip
                            dnp = ps2.tile([QH, 512], F32, tag="sc",
                                           name="dn", bufs=2)
                            for h in range(QH):
                                nc.tensor.matmul(
                                    dnp[:], onc4_sb[h][:], s16h[h][:],
                                    start=(h == 0), stop=(h == QH - 1),
                                    skip_group_check=True)
                            dn_sb = sm_pool.tile([QH, 512], F32, tag="dn_sb")
                            nc.vector.tensor_copy(dn_sb[:], dnp[:])
                            rc4 = sm_pool.tile([QH, 512], F32, tag="rc4")
                            scr = sm_pool.tile([QH, 512], F32, tag="scr")
                            nc.vector.reciprocal_approx_fast(rc4[:],
                                                             dn_sb[:])
                            rc16 = sm_pool.tile([QH, 512], BF, tag="rc16")
                            nc.vector.tensor_copy(rc16[:], rc4[:])
                            for h in range(QH):
                                dnr = sm_pool.tile([1, 512], BF, tag="dnr",
                                                   bufs=4)
                                nc.sync.dma_start(dnr[:], rc16[h:h + 1, :])
                                dnrs.append(dnr)

                        def norm_b():
                            for h in range(QH):
                                bcp = ps2.tile([P, 512], F32, tag="sc",
                                               name="bcp", bufs=2)
                                nc.tensor.matmul(bcp[:], onr_sb[:],
                                                 dnrs[h][:],
                                                 start=True, stop=True)
                                bc_sb = sm_pool.tile([P, 512], F32,
                                                     tag="bc_sb", bufs=3)
                                nc.vector.tensor_copy(bc_sb[:], bcp[:])
                                avn = sm_pool.tile([P, 512], BF, tag="avn",
                                                   bufs=4)
                                nc.vector.tensor_mul(avn[:], avcp[h][:],
                                                     bc_sb[:])
                                nc.sync.dma_start(
                                    ag_in[c, h * P:(h + 1) * P, :], avn[:])
                            nc.gpsimd.collective_compute(
                                "AllGather",
                                mybir.AluOpType.bypass,
                                replica_groups=[list(range(CORES))],
                                ins=[ag_in[c]],
                                outs=[ag_out[c]],
                            )
                        return norm_a, norm_b

                    # software pipeline: scores one j ahead; previous chunk's
                    # normalization + AllGather fire early; o_proj operand
                    # prefetch slots into the big attention chunk
                    scores(0)
                    for j in range(jmax + 1):
                        if j < jmax:
                            scores(j + 1)
                        if j == 0 and norm_pending is not None:
                            norm_pending[0]()
                        if j == 2 and norm_pending is not None:
                            norm_pending[1]()
                        if c == 3 and j == 6:
                            ag_tiles[1] = load_ag(1)
                        if c == 3 and j == 13:
                            ag_tiles[2] = load_ag(2)
                        if c == 0 and j == 3:
                            ag_tiles[3] = load_ag(3)
                        avdn(j)
                    norm_pending = make_norm()
                norm_pending[0]()
                mm_oproj(1, ag_tiles[1], ms=[0])   # dnr DMAs land meanwhile
                norm_pending[1]()          # triggers AG(0)
                mm_oproj(1, ag_tiles[1], ms=[1, 2, 3])
                mm_oproj(2, ag_tiles[2])
                mm_oproj(3, ag_tiles[3])
                ag_tiles[0] = load_ag(0)
                mm_oproj(0, ag_tiles[0])

            pstack.close()

    nc.finalize()
    return nc


_NC_CACHE = None


def _host_prep(positions, hidden_states, w_qkv, w_o):
    pos = np.asarray(positions).astype(np.float64)
    half = D // 2
    inv_freq = 1.0 / (THETA ** (np.arange(half, dtype=np.float64) * 2.0 / D))
    freqs = pos[:, None] * inv_freq[None, :]          # [T, 64]
    cos = np.cos(freqs).astype(np.float32).T          # [64, T]
    sin = np.sin(freqs).astype(np.float32).T
    # pair-interleaved basis: new row 2i <- old i, new row 2i+1 <- old 64+i
    cosT = np.empty((P, T), dtype=np.float32)
    sinT = np.empty((P, T), dtype=np.float32)
    cosT[0::2] = cos
    cosT[1::2] = cos
    sinT[0::2] = -sin
    sinT[1::2] = sin
    cosT = cosT.astype(BF_NP)
    sinT = sinT.astype(BF_NP)
    hidT = np.ascontiguousarray(
        np.asarray(hidden_states, dtype=np.float32).T).astype(BF_NP)
    w_qkv = np.asarray(w_qkv, dtype=np.float32)
    w_o = np.asarray(w_o, dtype=np.float32)
    perm = np.empty(D, dtype=np.int64)
    perm[0::2] = np.arange(half)
    perm[1::2] = np.arange(half) + half

    in_maps = []
    for c in range(CORES):
        q_rows = w_qkv[c * QH * D:(c + 1) * QH * D].reshape(QH, D, HIDDEN)
        q_rows = np.ascontiguousarray(q_rows[:, perm, :]).reshape(
            QH * D, HIDDEN)
        rows = [
            w_qkv[H * D + c * D: H * D + (c + 1) * D][perm],            # k
            w_qkv[(H + KV) * D + c * D: (H + KV) * D + (c + 1) * D],    # v
            q_rows,                                                     # q0..3
        ]
        wcat = np.concatenate(rows, axis=0)             # [768, 4096] (out,in)
        # stationary-major: [m*128+p, kc*128+cc] = wcat[m*128+cc, kc*128+p]
        w_qkvT_c = np.ascontiguousarray(
            wcat.reshape(SLOTS, P, KCH, P).transpose(0, 3, 2, 1)
            .reshape(SLOTS * P, KCH * P)).astype(BF_NP)
        wo_slice = w_o[c * LOCAL:(c + 1) * LOCAL, :]    # [512 out, 4096 in]
        w_oT_c = np.ascontiguousarray(
            wo_slice.reshape(QH, P, KCH, P).transpose(0, 3, 2, 1)
            .reshape(QH * P, KCH * P)).astype(BF_NP)
        in_maps.append({
            "hidT": hidT,
            "w_qkvT": w_qkvT_c,
            "w_oT": w_oT_c,
            "cosT": cosT,
            "sinT": sinT,
        })
    return in_maps


def kernel(positions, hidden_states, w_qkv, w_o):
    global _NC_CACHE
    in_maps = _host_prep(positions, hidden_states, w_qkv, w_o)
    if _NC_CACHE is None:
        _NC_CACHE = build_nc()
    res = None
    for attempt in range(3):
        try:
            res = run_bass_kernel_spmd(_NC_CACHE, in_maps,
                                       core_ids=list(range(CORES)))
            break
        except Exception:
            if attempt == 2:
                raise
    outs = [res.results[c]["outT"].T for c in range(CORES)]   # [2048, 512]
    return np.ascontiguousarray(np.concatenate(outs, axis=1))

